# revision 45
# baseline (speedup 1.0000x reference)
"""Multi-head attention (B=4, N=2048, D=1024, H=16) on 8 TRN2 NeuronCores.

Sharding: core c = (batch b = c // 2, head-group hg = c % 2). Each core:
  - computes Q/K/V for its 8 heads (tensor-parallel slice of qkv_w),
  - runs attention for those heads,
  - computes a partial output projection against its 512 columns of proj_w.
Host sums the two partials per batch and adds biases folded on the host.

The ENTIRE kernel runs in the PE's 64x64 array-packing mode (4 concurrent
tiles T0/T2/T8/T10), so HD=64 attention matmuls get ~2x PE throughput and
there are no tiling-mode switches (which drain the array).  bass derives
tile_position from (lhsT.base_partition, out.base_partition) and tile_size
from the AP shapes, so a matmul is a 64x64 tile op iff its lhsT/rhs use 64
partitions and its out covers <=64 partitions.

Device layouts (feature-on-partition; scores come out as S^T [k, q]):
  xt  [128, 8, 2048]  bf16 : x[b]^T, d = kt*128 + p
  wqk [128, 8, 1024]  bf16 : Q (slots 0..3) and K (slots 4..7) weights; the
                             slot t covers head pair (2t, 2t+1): partition
                             range 0:64 = even head dims, 64:128 = odd.
  wv  [128, 8, 512]   bf16 : rhs for V (token-on-partition orientation)
  wp  [128, 4, 1024]  bf16 : lhsT-side contraction layout for the proj
  bqk [128, 8]        f32  : per-feature q/k bias (zero in practice)
  out [2048, 1024]    f32  : partial projection output

Attention per (pair t, q-block qb, key-tile kt), all 64x64 tile ops:
  scores: 4 tiles (head x key-half) -> spA = even S^T [128k, 512q],
          spB = odd S^T.  K=64 per tile (the head dim) -> full PE use.
  exp:    ACT does a fixed 448-column slice (true exp); the DVE does the
          remaining 576 columns with a Schraudolph int16 fast-exp (writing
          the bf16 bit pattern of exp directly).  The column split is fixed
          across kt, so every (query, head) row is normalized by a
          denominator built from the same approximation - the systematic
          part of the fast-exp error cancels in the softmax division.
  AV:     4 tiles (head x k-half) accumulating over kt into avA (k-half 0)
          and avB (k-half 1); summed at evacuation.
  denom:  4 tiles multiplying a [64, 33] ones-column lhsT (M=33 keeps
          tile_size at 64x64) against the same P^T tiles, accumulating the
          softmax denominators in rows 0 / 64 of dA/dB.

Normalization: denominators bounce through DRAM (2 accumulating DMAs), come
back as a 128-partition broadcast, and gpsimd divides (avA+avB) by them into
o_sb.  No DVE reciprocal anywhere near the critical path; the DVE only does
exp plus the proj-group evacuations.

Feed work (QKV projections, V tiles, output-proj groups) is split 4-ways
into the same 64x64 mode (T0/T2/T8/T10 with contraction halves into a psA /
psB pair, summed+biased by gpsimd at evacuation) and issued into the
attention stream by the same (cycle, group) deadline schedule as before.

PSUM budget (8 banks): sp pair 2 + av pair 2 + denom pair 2 + feed pair 2.
The single-buffered score pair works because exp(kt) finishes (ACT 448 cols
~ 430ns in parallel with DVE 576 cols ~ 350ns) before the PE finishes the
rest of the kt cycle (AV + denom, ~430ns), so scores(kt+1) is never gated.
"""

import numpy as np
import ml_dtypes

import concourse.tile as tile
from concourse import bacc, mybir
from concourse._compat import with_exitstack

B, N, D, H, HD = 4, 2048, 1024, 16, 64
NCORES = 8
HPC = 8          # heads per core
FPC = HPC * HD   # 512 features per core
KT = 8           # d-contraction tiles of 128
KTT = 16         # key-token tiles of 128
QB = 512         # q-block size
NQB = N // QB
SCALE = HD ** -0.5

F32 = mybir.dt.float32
BF16 = mybir.dt.bfloat16
I16 = mybir.dt.int16
EXP = mybir.ActivationFunctionType.Exp
IDENT = mybir.ActivationFunctionType.Identity
ADD = mybir.AluOpType.add
MULT = mybir.AluOpType.mult
DIV = mybir.AluOpType.divide

# Schraudolph fast-exp on the DVE: bf16 bit pattern of exp(x*SCALE) is
# approximately round(A*x + B) as int16 (linear-mantissa approximation,
# ~1.8% rms / ~4.2% max relative error at the rms-optimal B).
EXPA = 128.0 * SCALE / float(np.log(2.0))   # 23.0835
EXPB = 16248.5
# Of each kt's 1024 P^T columns (512 even + 512 odd head), the ACT engine
# computes this many with the real exp; the DVE fast-exp does the rest.
ACT_COLS = 448


@with_exitstack
def _attn_body(ctx, tc, xt_d, wqk_d, wv_d, wp_d, bqk_d, out_d):
    nc = tc.nc

    singles = ctx.enter_context(tc.tile_pool(name="singles", bufs=1))
    evac = ctx.enter_context(tc.tile_pool(name="evac", bufs=4))
    ppool = ctx.enter_context(tc.tile_pool(name="ppool", bufs=3))
    rpool = ctx.enter_context(tc.tile_pool(name="rpool", bufs=3))
    dpool = ctx.enter_context(tc.tile_pool(name="dpool", bufs=8, space="DRAM"))
    ps_s = ctx.enter_context(tc.tile_pool(name="ps_s", bufs=1, space="PSUM"))
    ps_avd = ctx.enter_context(tc.tile_pool(name="ps_avd", bufs=1, space="PSUM"))

    # Warm the ACT exp table at t~0 so the ~2.7us ACT_TABLE_LOAD overlaps
    # the input DMAs instead of delaying the first real exp.
    warm = singles.tile([1, 1], F32, name="act_warm")
    nc.vector.memset(warm, 0.0)
    nc.scalar.activation(warm, warm, EXP)

    # Resident SBUF tensors.  wqk first (gates the first K/Q projections),
    # split into two kt-halves so the first projection matmuls start after
    # 1MB; then x token-quarters so dependent work starts per-quarter.
    wqk_h = []
    for h in range(2):
        t = singles.tile([128, KT // 2, 2 * FPC], BF16, name=f"wqk_h{h}")
        wqk_h.append(t)
    nc.sync.dma_start(wqk_h[0], wqk_d[:, 0:KT // 2, :])
    xt_q = []
    for q in range(4):
        t = singles.tile([128, KT, 512], BF16, name=f"xt_q{q}")
        xt_q.append(t)
    nc.sync.dma_start(xt_q[0], xt_d[:, :, 0:512])
    nc.sync.dma_start(wqk_h[1], wqk_d[:, KT // 2:KT, :])
    bqk_sb = singles.tile([128, 8], F32)
    nc.sync.dma_start(bqk_sb, bqk_d[:])
    wv_sb = singles.tile([128, KT, FPC], BF16)
    nc.sync.dma_start(wv_sb, wv_d[:])
    for q in range(1, 4):
        nc.sync.dma_start(xt_q[q], xt_d[:, :, 512 * q:512 * (q + 1)])
    wp_sb = singles.tile([128, 4, D], BF16)
    nc.sync.dma_start(wp_sb, wp_d[:])

    def wqk_kt(kt, f0, width):
        return wqk_h[kt // 4][:, kt % 4, f0:f0 + width]

    def xt_tok(kt, c0, width):
        """Token-range slice of x^T across the quarters (never spans)."""
        q, off = divmod(c0, 512)
        assert off + width <= 512
        return xt_q[q][:, kt, off:off + width]

    qk_sb = singles.tile([128, 8, N], BF16)        # Q^T slots 0..3, K^T slots 4..7
    v_sb = singles.tile([128, KTT, HPC, HD], BF16)  # V, token-on-partition
    o_sb = singles.tile([128, 4, N], BF16)         # normalized attn out
    # ones-column lhsT for the softmax-denominator matmuls: col 0 is 1.0,
    # cols 1..32 are 0 (M=33 so the tile_size stays 64x64).
    ones_sb = singles.tile([128, 33], BF16, name="ones33")
    nc.vector.memset(ones_sb, 0.0)
    nc.vector.memset(ones_sb[:, 0:1], 1.0)

    def emit_qk(ft, qt, pool, tag, on_act=False):
        """One (ft, qt) group of the Q/K projection, full 128x128 mode: 8
        accumulating K=128 matmuls into one PSUM bank + a single bias-fused
        evacuation straight into qk_sb."""
        ps = pool.tile([128, 512], F32, tag=tag, name="qk_ps")
        f0 = ft * 128
        for kt in range(KT):
            nc.tensor.matmul(ps, wqk_kt(kt, f0, 128),
                             xt_tok(kt, qt * 512, 512),
                             start=(kt == 0), stop=(kt == KT - 1))
        dst = qk_sb[:, ft, qt * 512:(qt + 1) * 512]
        if on_act:
            nc.scalar.activation(dst, ps, IDENT, bias=bqk_sb[:, ft:ft + 1])
        else:
            nc.vector.tensor_scalar(dst, ps, bqk_sb[:, ft:ft + 1], None,
                                    op0=ADD)

    def emit_v(mt, pool, tag, on_act=False):
        """One token-tile of the V projection, full 128x128 mode, with a
        single strided evacuation into v_sb."""
        ps = pool.tile([128, 512], F32, tag=tag, name="v_ps")
        for kt in range(KT):
            nc.tensor.matmul(ps, xt_tok(kt, mt * 128, 128),
                             wv_sb[:, kt, :],
                             start=(kt == 0), stop=(kt == KT - 1))
        src = ps.rearrange("p (h e) -> p h e", h=HPC)
        if on_act:
            nc.scalar.copy(v_sb[:, mt, :, :], src)
        else:
            nc.vector.tensor_copy(v_sb[:, mt, :, :], src)

    def emit_proj(mt, et, pool, tag, on_act=False):
        """One output-projection group (tail only, full 128x128 mode): 4
        accumulating K=128 matmuls into a single PSUM bank + copy + store."""
        ps = pool.tile([128, 512], F32, tag=tag, name="pj_ps")
        m0 = mt * 128
        e0 = et * 512
        order = (3, 0, 1, 2)
        for i, t4 in enumerate(order):
            nc.tensor.matmul(ps, o_sb[:, t4, m0:m0 + 128],
                             wp_sb[:, t4, e0:e0 + 512],
                             start=(i == 0), stop=(i == 3))
        ot = evac.tile([128, 512], F32, tag="oevac", name="o_evac")
        if on_act:
            nc.scalar.copy(ot, ps)
        else:
            nc.vector.tensor_copy(ot, ps)
        nc.sync.dma_start(out_d[m0:m0 + 128, e0:e0 + 512], ot)

    # Normalization.  stage1 (at block end): exit the merged AV+denominator
    # pair from PSUM with one big copy per tile (DVE for A, ACT for B) and
    # bounce the 4 denominator partial rows through DRAM into a 128-lane
    # layout.  stage2 (mid next block): reciprocal, broadcast back, and
    # gpsimd combines (afA + afB) * (1/d) into o_sb.
    def normalize_stage1(t, qb, avdA, avdB):
        afA = rpool.tile([128, 2 * QB], F32, tag="afA", name="afA_t")
        afB = rpool.tile([128, 2 * QB], F32, tag="afB", name="afB_t")
        nc.vector.tensor_copy(afA, avdA)
        nc.scalar.copy(afB, avdB)
        rd = dpool.tile([4, QB], F32, name="d_dram")
        nc.sync.dma_start(rd[0:1, :], afA[0:1, QB:2 * QB])
        nc.sync.dma_start(rd[1:2, :], afA[64:65, QB:2 * QB])
        nc.sync.dma_start(rd[2:3, :], afB[0:1, QB:2 * QB])
        nc.sync.dma_start(rd[3:4, :], afB[64:65, QB:2 * QB])
        d128 = rpool.tile([128, 4, 4], F32, tag="d128", name="d128_t")
        nc.sync.dma_start(
            d128, rd[:].rearrange("r (a p) -> p r a", p=128))
        return (t, qb, afA, afB, d128)

    def normalize_stage2(st):
        t, qb, afA, afB, d128 = st
        q0 = qb * QB
        # d = dA + dB (r=0,1 are even/odd from the A half; r=2,3 from B)
        dsum = rpool.tile([128, 2, 4], F32, tag="dsum", name="dsum_t")
        nc.vector.tensor_tensor(dsum, d128[:, 0:2, :], d128[:, 2:4, :], op=ADD)
        r128 = rpool.tile([128, 2, 4], F32, tag="r128", name="r128_t")
        nc.vector.reciprocal(r128, dsum)
        rr = dpool.tile([2, QB], F32, name="r_dram")
        nc.sync.dma_start(rr[:].rearrange("r (a p) -> p r a", p=128), r128)
        rb = rpool.tile([128, QB], F32, tag="rb", name="rb_t")
        nc.sync.dma_start(rb[0:64, :], rr[0:1, :].partition_broadcast(64))
        nc.sync.dma_start(rb[64:128, :], rr[1:2, :].partition_broadcast(64))
        af = rpool.tile([128, QB], F32, tag="af", name="af_t")
        nc.gpsimd.tensor_add(af, afA[:, 0:QB], afB[:, 0:QB])
        nc.gpsimd.tensor_mul(o_sb[:, t, q0:q0 + QB], af, rb)



    # Deadline-scheduled feeds: per pair, a list of (cycle, group) issued
    # into the attention stream once the (qb*KTT + kt) cycle counter reaches
    # `cycle`.  Deadlines: V(kt) <= kt (AV of block 0 consumes it), K(4,j)
    # <= 4j-1, Q(0,qb) <= 16qb-1, pair-(t+1) slots <= 63.
    feeds = [
        [(0, ("v", 0)), (0, ("v", 1)), (1, ("v", 2)), (2, ("v", 3)),
         (2, ("qk", 4, 1)), (3, ("v", 4)), (4, ("v", 5)), (5, ("v", 6)),
         (6, ("v", 7)), (6, ("qk", 4, 2)), (7, ("v", 8)), (8, ("v", 9)),
         (9, ("v", 10)), (10, ("v", 11)), (10, ("qk", 4, 3)),
         (11, ("v", 12)), (12, ("v", 13)), (13, ("v", 14)),
         (13, ("qk", 0, 1)), (14, ("v", 15)),
         (16, ("qk", 0, 2)), (18, ("qk", 5, 0)), (20, ("qk", 5, 1)),
         (22, ("qk", 5, 2)), (24, ("qk", 5, 3)), (26, ("qk", 1, 0)),
         (28, ("qk", 1, 1)), (30, ("qk", 1, 2)), (32, ("qk", 1, 3)),
         (34, ("qk", 0, 3))],
        [(4 * i, ("qk", f, qt))
         for i, (f, qt) in enumerate((f, qt) for qt in range(4) for f in (2, 6))]
        + [(30 + 4 * i, g) for i, g in enumerate(
            [("qk", 7, qt) for qt in range(4)]
            + [("qk", 3, qt) for qt in range(4)])],
        [],
        [],
    ]
    for t in range(4):
        feeds[t] = list(feeds[t])

    # ---- Attention: one flat software pipeline over (t, qb, kt).
    # AV + denominator run one kt behind scores/exp; each block's stage1 is
    # emitted right after its last AV, stage2 at cycle 1 of the next block.
    blocks = [(t, qb) for t in range(4) for qb in range(NQB)]
    # AV/denominator work runs AVLAG kts behind scores/exp so its exp
    # dependency is long-satisfied when the PE reaches it (no PE idle gap
    # between scores(k+1) and AV(k)).
    AVLAG = 2
    pending = []    # deque of (t, kt, avdA, avdB, ptA, ptB, start, fin, qb)
    norm_q = []     # deferred normalize stage-2 states

    def flush_pending(limit):
        while len(pending) > limit:
            flush_one(pending.pop(0))

    def flush_one(p):
        t, kt, avdA, avdB, ptA, ptB, st, fin, qb = p
        nc.tensor.matmul(avdA[0:64, 0:QB], v_sb[0:64, kt, 2 * t, :],
                         ptA[0:64, :], start=st, stop=fin)
        nc.tensor.matmul(avdA[64:128, 0:QB], v_sb[0:64, kt, 2 * t + 1, :],
                         ptB[0:64, :], start=st, stop=fin)
        nc.tensor.matmul(avdB[0:64, 0:QB], v_sb[64:128, kt, 2 * t, :],
                         ptA[64:128, :], start=st, stop=fin)
        nc.tensor.matmul(avdB[64:128, 0:QB], v_sb[64:128, kt, 2 * t + 1, :],
                         ptB[64:128, :], start=st, stop=fin)
        nc.tensor.matmul(avdA[0:33, QB:2 * QB], ones_sb[0:64, :],
                         ptA[0:64, :], start=st, stop=fin)
        nc.tensor.matmul(avdA[64:97, QB:2 * QB], ones_sb[0:64, :],
                         ptB[0:64, :], start=st, stop=fin)
        nc.tensor.matmul(avdB[0:33, QB:2 * QB], ones_sb[64:128, :],
                         ptA[64:128, :], start=st, stop=fin)
        nc.tensor.matmul(avdB[64:97, QB:2 * QB], ones_sb[64:128, :],
                         ptB[64:128, :], start=st, stop=fin)
        if fin:
            norm_q.append(normalize_stage1(t, qb, avdA, avdB))

    def run_pairs(pair_list, sp_sets, feed_pool):
        for t in pair_list:
            for qb in range(NQB):
                q0 = qb * QB
                avdA = ps_avd.tile([128, 2 * QB], F32, tag="avdA",
                                   name="avdA_t")
                avdB = ps_avd.tile([128, 2 * QB], F32, tag="avdB",
                                   name="avdB_t")
                for kt in range(KTT):
                    k0 = kt * 128
                    cyc = qb * KTT + kt
                    # stage2 lags two blocks so the DRAM bounce of the
                    # denominators (a ~2000-descriptor transposing gather,
                    # 10us+) never head-of-line-blocks the DVE queue.
                    if cyc % KTT == 8 and len(norm_q) >= 2:
                        normalize_stage2(norm_q.pop(0))
                    pool_s, ta, tb = sp_sets[kt % len(sp_sets)]
                    spA = pool_s.tile([128, 512], F32, tag=ta, name="spA_t")
                    spB = pool_s.tile([128, 512], F32, tag=tb, name="spB_t")
                    # scores: even head -> spA (keys on partitions), odd ->
                    # spB
                    nc.tensor.matmul(
                        spA[0:64, :], qk_sb[0:64, 4 + t, k0:k0 + 64],
                        qk_sb[0:64, t, q0:q0 + 512], start=True, stop=True)
                    nc.tensor.matmul(
                        spA[64:128, :], qk_sb[0:64, 4 + t, k0 + 64:k0 + 128],
                        qk_sb[0:64, t, q0:q0 + 512], start=True, stop=True)
                    nc.tensor.matmul(
                        spB[0:64, :], qk_sb[64:128, 4 + t, k0:k0 + 64],
                        qk_sb[64:128, t, q0:q0 + 512], start=True, stop=True)
                    nc.tensor.matmul(
                        spB[64:128, :], qk_sb[64:128, 4 + t, k0 + 64:k0 + 128],
                        qk_sb[64:128, t, q0:q0 + 512], start=True, stop=True)
                    ptA = ppool.tile([128, 512], BF16, tag="ptA", name="ptA_t")
                    ptB = ppool.tile([128, 512], BF16, tag="ptB", name="ptB_t")
                    # exp: ACT does the even head (true exp), the DVE fast-
                    # exp does the odd head.  One instruction per engine per
                    # kt - the per-instruction overheads are large.  The
                    # assignment is fixed so every (query, head) softmax row
                    # is built from one consistent approximation.
                    nc.scalar.activation(ptA, spA, EXP, scale=SCALE)
                    nc.vector.tensor_scalar(
                        ptB[:].bitcast(I16), spB[:],
                        EXPA, EXPB, op0=MULT, op1=ADD)
                    pending.append((t, kt, avdA, avdB, ptA, ptB,
                                    kt == 0, kt == KTT - 1, qb))
                    flush_pending(AVLAG)
                    while feeds[t] and feeds[t][0][0] <= cyc:
                        _, g = feeds[t].pop(0)
                        nfeed[0] += 1
                        tag = "wA" if nfeed[0] % 2 == 0 else "wB"
                        on_act = nfeed[0] % 2 == 1
                        if g[0] == "qk":
                            emit_qk(g[1], g[2], feed_pool, tag, on_act)
                        else:
                            emit_v(g[1], feed_pool, tag, on_act)

    # Pairs 0-1 carry ALL the feed work (their kt cycles are PE-bound, so
    # the exp chain hides inside the feed streams); the feed PSUM pool is
    # scoped so its banks are reclaimed afterwards.
    nfeed = [0]
    with tc.tile_pool(name="ps_w", bufs=1, space="PSUM") as ps_w:
        emit_qk(4, 0, ps_w, "wA")
        emit_qk(0, 0, ps_w, "wB", on_act=True)
        run_pairs([0, 1], [(ps_s, "spA", "spB")], ps_w)

    # Pairs 2-3 are feed-free and would be serialized on exp with a single
    # score-PSUM pair; the two banks freed above double-buffer the scores
    # (alternate bank pairs by kt parity) so exp(kt) overlaps scores(kt+1).
    with tc.tile_pool(name="ps_s2", bufs=1, space="PSUM") as ps_s2:
        run_pairs([2, 3],
                  [(ps_s, "spA", "spB"), (ps_s2, "spA2", "spB2")], None)
        flush_pending(0)
        while norm_q:
            normalize_stage2(norm_q.pop(0))

        # ---- Tail: ALL output-projection groups, full 128x128 mode (no
        # PSUM pair needed, one mode switch total), 4-deep psum rotation so
        # group i+1's matmuls overlap group i's evacuation, copies
        # alternating between ACT and DVE.
        tail = [(mt, et) for mt in range(KTT) for et in range(2)]
        tail_pools = [(ps_s, "spA"), (ps_s2, "spA2"),
                      (ps_s, "spB"), (ps_s2, "spB2")]
        for i, (mt, et) in enumerate(tail):
            pool, tag = tail_pools[i % 4]
            emit_proj(mt, et, pool, tag, on_act=(i % 2 == 1))


def build_nc():
    nc = bacc.Bacc()
    xt = nc.declare_dram_parameter("xt", [128, KT, N], BF16, isOutput=False)
    wqk = nc.declare_dram_parameter("wqk", [128, KT, 2 * FPC], BF16, isOutput=False)
    wv = nc.declare_dram_parameter("wv", [128, KT, FPC], BF16, isOutput=False)
    wp = nc.declare_dram_parameter("wp", [128, 4, D], BF16, isOutput=False)
    bqk = nc.declare_dram_parameter("bqk", [128, 8], F32, isOutput=False)
    out = nc.declare_dram_parameter("out", [N, D], F32, isOutput=True)
    with tile.TileContext(nc) as tc:
        _attn_body(tc, xt, wqk, wv, wp, bqk, out)
    nc.finalize()
    return nc


BF = ml_dtypes.bfloat16


def prep_core_inputs(x, qkv_w, qkv_b, proj_w, c):
    """Build the per-core input map (numpy, final SBUF layouts)."""
    b, hg = divmod(c, 2)
    f0 = hg * FPC
    xt = np.ascontiguousarray(x[b].T)                     # [1024, 2048] f32
    xt_sb = xt.reshape(KT, 128, N).transpose(1, 0, 2)     # [128, 8, 2048]
    wq = qkv_w[f0:f0 + FPC]
    wk = qkv_w[D + f0:D + f0 + FPC]
    wqk = np.concatenate([wq, wk], axis=0)                # [1024, 1024]
    wqk_sb = wqk.T.reshape(KT, 128, 2 * FPC).transpose(1, 0, 2)
    wv = qkv_w[2 * D + f0:2 * D + f0 + FPC]               # [512, 1024]
    wv_sb = wv.T.reshape(KT, 128, FPC).transpose(1, 0, 2)
    wp = proj_w[:, f0:f0 + FPC]                           # [1024e, 512f]
    wp_sb = wp.T.reshape(4, 128, D).transpose(1, 0, 2)
    bqk = np.concatenate(
        [qkv_b[f0:f0 + FPC], qkv_b[D + f0:D + f0 + FPC]]).reshape(8, 128).T
    return {
        "xt": np.ascontiguousarray(xt_sb).astype(BF),
        "wqk": np.ascontiguousarray(wqk_sb).astype(BF),
        "wv": np.ascontiguousarray(wv_sb).astype(BF),
        "wp": np.ascontiguousarray(wp_sb).astype(BF),
        "bqk": np.ascontiguousarray(bqk).astype(np.float32),
    }


def expected_core_out(x, qkv_w, qkv_b, proj_w, c):
    """Numpy model of one core's partial output (for sim debugging)."""
    b, hg = divmod(c, 2)
    f0 = hg * FPC
    xb = x[b].astype(np.float32)
    q = xb @ qkv_w[f0:f0 + FPC].T + qkv_b[f0:f0 + FPC]
    k = xb @ qkv_w[D + f0:D + f0 + FPC].T + qkv_b[D + f0:D + f0 + FPC]
    v = xb @ qkv_w[2 * D + f0:2 * D + f0 + FPC].T          # v-bias folded on host
    out = np.zeros((N, D), np.float32)
    for h in range(HPC):
        qs = q[:, h * HD:(h + 1) * HD]
        ks = k[:, h * HD:(h + 1) * HD]
        vs = v[:, h * HD:(h + 1) * HD]
        s = (qs @ ks.T) * SCALE
        p = np.exp(s - s.max(axis=1, keepdims=True))
        p /= p.sum(axis=1, keepdims=True)
        out += (p @ vs) @ proj_w[:, f0 + h * HD:f0 + (h + 1) * HD].T
    return out


_NC_CACHE = {}


def kernel(x, qkv_w, qkv_b, proj_w, proj_b):
    from concourse.bass_utils import run_bass_kernel_spmd

    x = np.asarray(x, dtype=np.float32)
    qkv_w = np.asarray(qkv_w, dtype=np.float32)
    qkv_b = np.asarray(qkv_b, dtype=np.float32)
    proj_w = np.asarray(proj_w, dtype=np.float32)
    proj_b = np.asarray(proj_b, dtype=np.float32)

    if "nc" not in _NC_CACHE:
        _NC_CACHE["nc"] = build_nc()
    nc = _NC_CACHE["nc"]

    in_maps = [
        prep_core_inputs(x, qkv_w, qkv_b, proj_w, c) for c in range(NCORES)
    ]
    res = run_bass_kernel_spmd(nc, in_maps, core_ids=list(range(NCORES)))
    outs = res.results

    # v-bias folds into a constant row added to every token: proj_w @ v_bias.
    const_row = proj_w @ qkv_b[2 * D:3 * D] + proj_b
    full = np.empty((B, N, D), np.float32)
    for b in range(B):
        full[b] = outs[2 * b]["out"] + outs[2 * b + 1]["out"] + const_row
    return full


# revision 54
# speedup vs baseline: 1.0371x; 1.0371x over previous
"""Multi-head attention (B=4, N=2048, D=1024, H=16) on 8 TRN2 NeuronCores.

Sharding: core c = (batch b = c // 2, head-group hg = c % 2). Each core:
  - computes Q/K/V for its 8 heads (tensor-parallel slice of qkv_w),
  - runs attention for those heads,
  - computes a partial output projection against its 512 columns of proj_w.
Host sums the two partials per batch and adds biases folded on the host.

The ENTIRE kernel runs in the PE's 64x64 array-packing mode (4 concurrent
tiles T0/T2/T8/T10), so HD=64 attention matmuls get ~2x PE throughput and
there are no tiling-mode switches (which drain the array).  bass derives
tile_position from (lhsT.base_partition, out.base_partition) and tile_size
from the AP shapes, so a matmul is a 64x64 tile op iff its lhsT/rhs use 64
partitions and its out covers <=64 partitions.

Device layouts (feature-on-partition; scores come out as S^T [k, q]):
  xt  [128, 8, 2048]  bf16 : x[b]^T, d = kt*128 + p
  wqk [128, 8, 1024]  bf16 : Q (slots 0..3) and K (slots 4..7) weights; the
                             slot t covers head pair (2t, 2t+1): partition
                             range 0:64 = even head dims, 64:128 = odd.
  wv  [128, 8, 512]   bf16 : rhs for V (token-on-partition orientation)
  wp  [128, 4, 1024]  bf16 : lhsT-side contraction layout for the proj
  bqk [128, 8]        f32  : per-feature q/k bias (zero in practice)
  out [2048, 1024]    f32  : partial projection output

Attention per (pair t, q-block qb, key-tile kt), all 64x64 tile ops:
  scores: 4 tiles (head x key-half) -> spA = even S^T [128k, 512q],
          spB = odd S^T.  K=64 per tile (the head dim) -> full PE use.
  exp:    ACT does a fixed 448-column slice (true exp); the DVE does the
          remaining 576 columns with a Schraudolph int16 fast-exp (writing
          the bf16 bit pattern of exp directly).  The column split is fixed
          across kt, so every (query, head) row is normalized by a
          denominator built from the same approximation - the systematic
          part of the fast-exp error cancels in the softmax division.
  AV:     4 tiles (head x k-half) accumulating over kt into avA (k-half 0)
          and avB (k-half 1); summed at evacuation.
  denom:  4 tiles multiplying a [64, 33] ones-column lhsT (M=33 keeps
          tile_size at 64x64) against the same P^T tiles, accumulating the
          softmax denominators in rows 0 / 64 of dA/dB.

Normalization: denominators bounce through DRAM (2 accumulating DMAs), come
back as a 128-partition broadcast, and gpsimd divides (avA+avB) by them into
o_sb.  No DVE reciprocal anywhere near the critical path; the DVE only does
exp plus the proj-group evacuations.

Feed work (QKV projections, V tiles, output-proj groups) is split 4-ways
into the same 64x64 mode (T0/T2/T8/T10 with contraction halves into a psA /
psB pair, summed+biased by gpsimd at evacuation) and issued into the
attention stream by the same (cycle, group) deadline schedule as before.

PSUM budget (8 banks): sp pair 2 + av pair 2 + denom pair 2 + feed pair 2.
The single-buffered score pair works because exp(kt) finishes (ACT 448 cols
~ 430ns in parallel with DVE 576 cols ~ 350ns) before the PE finishes the
rest of the kt cycle (AV + denom, ~430ns), so scores(kt+1) is never gated.
"""

import numpy as np
import ml_dtypes

import concourse.tile as tile
from concourse import bacc, mybir
from concourse._compat import with_exitstack

B, N, D, H, HD = 4, 2048, 1024, 16, 64
NCORES = 8
HPC = 8          # heads per core
FPC = HPC * HD   # 512 features per core
KT = 8           # d-contraction tiles of 128
KTT = 16         # key-token tiles of 128
QB = 512         # q-block size
NQB = N // QB
SCALE = HD ** -0.5

F32 = mybir.dt.float32
BF16 = mybir.dt.bfloat16
I16 = mybir.dt.int16
EXP = mybir.ActivationFunctionType.Exp
IDENT = mybir.ActivationFunctionType.Identity
ADD = mybir.AluOpType.add
MULT = mybir.AluOpType.mult
DIV = mybir.AluOpType.divide

# Schraudolph fast-exp on the DVE: bf16 bit pattern of exp(x*SCALE) is
# approximately round(A*x + B) as int16 (linear-mantissa approximation,
# ~1.8% rms / ~4.2% max relative error at the rms-optimal B).
EXPA = 128.0 * SCALE / float(np.log(2.0))   # 23.0835
EXPB = 16248.5
# Of each kt's 1024 P^T columns (512 even + 512 odd head), the ACT engine
# computes this many with the real exp; the DVE fast-exp does the rest.
ACT_COLS = 448


@with_exitstack
def _attn_body(ctx, tc, xt_d, wqk_d, wv_d, wp_d, bqk_d, out_d):
    nc = tc.nc

    singles = ctx.enter_context(tc.tile_pool(name="singles", bufs=1))
    evac = ctx.enter_context(tc.tile_pool(name="evac", bufs=4))
    ppool = ctx.enter_context(tc.tile_pool(name="ppool", bufs=5))
    rpool = ctx.enter_context(tc.tile_pool(name="rpool", bufs=3))
    dpool = ctx.enter_context(tc.tile_pool(name="dpool", bufs=8, space="DRAM"))
    ps_s = ctx.enter_context(tc.tile_pool(name="ps_s", bufs=1, space="PSUM"))
    ps_avd = ctx.enter_context(tc.tile_pool(name="ps_avd", bufs=1, space="PSUM"))

    # Warm the ACT exp table at t~0 so the ~2.7us ACT_TABLE_LOAD overlaps
    # the input DMAs instead of delaying the first real exp.
    warm = singles.tile([1, 1], F32, name="act_warm")
    nc.vector.memset(warm, 0.0)
    nc.scalar.activation(warm, warm, EXP)

    # Resident SBUF tensors.  wqk first (gates the first K/Q projections),
    # split into two kt-halves so the first projection matmuls start after
    # 1MB; then x token-quarters so dependent work starts per-quarter.
    wqk_h = []
    for h in range(2):
        t = singles.tile([128, KT // 2, 2 * FPC], BF16, name=f"wqk_h{h}")
        wqk_h.append(t)
    nc.sync.dma_start(wqk_h[0], wqk_d[:, 0:KT // 2, :])
    xt_q = []
    for q in range(4):
        t = singles.tile([128, KT, 512], BF16, name=f"xt_q{q}")
        xt_q.append(t)
    nc.sync.dma_start(xt_q[0], xt_d[:, :, 0:512])
    nc.sync.dma_start(wqk_h[1], wqk_d[:, KT // 2:KT, :])
    bqk_sb = singles.tile([128, 8], F32)
    nc.sync.dma_start(bqk_sb, bqk_d[:])
    wv_sb = singles.tile([128, KT, FPC], BF16)
    nc.sync.dma_start(wv_sb, wv_d[:])
    for q in range(1, 4):
        nc.sync.dma_start(xt_q[q], xt_d[:, :, 512 * q:512 * (q + 1)])
    wp_sb = singles.tile([128, 4, D], BF16)
    nc.sync.dma_start(wp_sb, wp_d[:])

    def wqk_kt(kt, f0, width):
        return wqk_h[kt // 4][:, kt % 4, f0:f0 + width]

    def xt_tok(kt, c0, width):
        """Token-range slice of x^T across the quarters (never spans)."""
        q, off = divmod(c0, 512)
        assert off + width <= 512
        return xt_q[q][:, kt, off:off + width]

    qk_sb = singles.tile([128, 8, N], BF16)        # Q^T slots 0..3, K^T slots 4..7
    v_sb = singles.tile([128, KTT, HPC, HD], BF16)  # V, token-on-partition
    o_sb = singles.tile([128, 4, N], BF16)         # normalized attn out
    # ones-column lhsT for the softmax-denominator matmuls: col 0 is 1.0,
    # cols 1..32 are 0 (M=33 so the tile_size stays 64x64).
    ones_sb = singles.tile([128, 33], BF16, name="ones33")
    nc.vector.memset(ones_sb, 0.0)
    nc.vector.memset(ones_sb[:, 0:1], 1.0)

    def emit_qk(ft, qt, pool, tag, on_act=False):
        """One (ft, qt) group of the Q/K projection, full 128x128 mode: 8
        accumulating K=128 matmuls into one PSUM bank + a single bias-fused
        evacuation straight into qk_sb."""
        ps = pool.tile([128, 512], F32, tag=tag, name="qk_ps")
        f0 = ft * 128
        for kt in range(KT):
            nc.tensor.matmul(ps, wqk_kt(kt, f0, 128),
                             xt_tok(kt, qt * 512, 512),
                             start=(kt == 0), stop=(kt == KT - 1))
        dst = qk_sb[:, ft, qt * 512:(qt + 1) * 512]
        if on_act:
            nc.scalar.activation(dst, ps, IDENT, bias=bqk_sb[:, ft:ft + 1])
        else:
            nc.vector.tensor_scalar(dst, ps, bqk_sb[:, ft:ft + 1], None,
                                    op0=ADD)

    def emit_v(mt, pool, tag, on_act=False):
        """One token-tile of the V projection, full 128x128 mode, with a
        single strided evacuation into v_sb."""
        ps = pool.tile([128, 512], F32, tag=tag, name="v_ps")
        for kt in range(KT):
            nc.tensor.matmul(ps, xt_tok(kt, mt * 128, 128),
                             wv_sb[:, kt, :],
                             start=(kt == 0), stop=(kt == KT - 1))
        src = ps.rearrange("p (h e) -> p h e", h=HPC)
        if on_act:
            nc.scalar.copy(v_sb[:, mt, :, :], src)
        else:
            nc.vector.tensor_copy(v_sb[:, mt, :, :], src)

    def emit_proj(mt, et, pool, tag, on_act=False):
        """One output-projection group (tail only, full 128x128 mode): 4
        accumulating K=128 matmuls into a single PSUM bank + copy + store."""
        ps = pool.tile([128, 512], F32, tag=tag, name="pj_ps")
        m0 = mt * 128
        e0 = et * 512
        order = (3, 0, 1, 2)
        for i, t4 in enumerate(order):
            nc.tensor.matmul(ps, o_sb[:, t4, m0:m0 + 128],
                             wp_sb[:, t4, e0:e0 + 512],
                             start=(i == 0), stop=(i == 3))
        ot = evac.tile([128, 512], F32, tag="oevac", name="o_evac")
        if on_act:
            nc.scalar.copy(ot, ps)
        else:
            nc.vector.tensor_copy(ot, ps)
        nc.sync.dma_start(out_d[m0:m0 + 128, e0:e0 + 512], ot)

    # Normalization.  stage1 (at block end): exit the merged AV+denominator
    # pair from PSUM with one big copy per tile (DVE for A, ACT for B) and
    # bounce the 4 denominator partial rows through DRAM into a 128-lane
    # layout.  stage2 (mid next block): reciprocal, broadcast back, and
    # gpsimd combines (afA + afB) * (1/d) into o_sb.
    def normalize_stage1(t, qb, avdA, avdB):
        afA = rpool.tile([128, 2 * QB], F32, tag="afA", name="afA_t")
        afB = rpool.tile([128, 2 * QB], F32, tag="afB", name="afB_t")
        # four half-size exits split across ACT and DVE so the banks free
        # ~2x sooner (the next block's AV start is gated on them)
        nc.vector.tensor_copy(afA[:, 0:QB], avdA[:, 0:QB])
        nc.scalar.copy(afA[:, QB:2 * QB], avdA[:, QB:2 * QB])
        nc.vector.tensor_copy(afB[:, QB:2 * QB], avdB[:, QB:2 * QB])
        nc.scalar.copy(afB[:, 0:QB], avdB[:, 0:QB])
        rd = dpool.tile([4, QB], F32, name="d_dram")
        nc.sync.dma_start(rd[0:1, :], afA[0:1, QB:2 * QB])
        nc.sync.dma_start(rd[1:2, :], afA[64:65, QB:2 * QB])
        nc.sync.dma_start(rd[2:3, :], afB[0:1, QB:2 * QB])
        nc.sync.dma_start(rd[3:4, :], afB[64:65, QB:2 * QB])
        d128 = rpool.tile([128, 4, 4], F32, tag="d128", name="d128_t")
        nc.sync.dma_start(
            d128, rd[:].rearrange("r (a p) -> p r a", p=128))
        return (t, qb, afA, afB, d128)

    def normalize_stage2(st):
        t, qb, afA, afB, d128 = st
        q0 = qb * QB
        # d = dA + dB (r=0,1 are even/odd from the A half; r=2,3 from B)
        dsum = rpool.tile([128, 2, 4], F32, tag="dsum", name="dsum_t")
        nc.vector.tensor_tensor(dsum, d128[:, 0:2, :], d128[:, 2:4, :], op=ADD)
        r128 = rpool.tile([128, 2, 4], F32, tag="r128", name="r128_t")
        nc.vector.reciprocal(r128, dsum)
        rr = dpool.tile([2, QB], F32, name="r_dram")
        nc.sync.dma_start(rr[:].rearrange("r (a p) -> p r a", p=128), r128)
        rb = rpool.tile([128, QB], F32, tag="rb", name="rb_t")
        nc.sync.dma_start(rb[0:64, :], rr[0:1, :].partition_broadcast(64))
        nc.sync.dma_start(rb[64:128, :], rr[1:2, :].partition_broadcast(64))
        af = rpool.tile([128, QB], F32, tag="af", name="af_t")
        nc.gpsimd.tensor_add(af, afA[:, 0:QB], afB[:, 0:QB])
        nc.gpsimd.tensor_mul(o_sb[:, t, q0:q0 + QB], af, rb)



    # Deadline-scheduled feeds: per pair, a list of (cycle, group) issued
    # into the attention stream once the (qb*KTT + kt) cycle counter reaches
    # `cycle`.  Deadlines: V(kt) <= kt (AV of block 0 consumes it), K(4,j)
    # <= 4j-1, Q(0,qb) <= 16qb-1, pair-(t+1) slots <= 63.
    feeds = [
        [(0, ("v", 0)), (0, ("v", 1)), (1, ("v", 2)), (2, ("v", 3)),
         (2, ("qk", 4, 1)), (3, ("v", 4)), (4, ("v", 5)), (5, ("v", 6)),
         (6, ("v", 7)), (6, ("qk", 4, 2)), (7, ("v", 8)), (8, ("v", 9)),
         (9, ("v", 10)), (10, ("v", 11)), (10, ("qk", 4, 3)),
         (11, ("v", 12)), (12, ("v", 13)), (13, ("v", 14)),
         (13, ("qk", 0, 1)), (14, ("v", 15)),
         (16, ("qk", 0, 2)), (18, ("qk", 5, 0)), (20, ("qk", 5, 1)),
         (22, ("qk", 5, 2)), (24, ("qk", 5, 3)), (26, ("qk", 1, 0)),
         (28, ("qk", 1, 1)), (30, ("qk", 1, 2)), (32, ("qk", 1, 3)),
         (34, ("qk", 0, 3))],
        [(4 * i, ("qk", f, qt))
         for i, (f, qt) in enumerate((f, qt) for qt in range(4) for f in (2, 6))]
        + [(30 + 4 * i, g) for i, g in enumerate(
            [("qk", 7, qt) for qt in range(4)]
            + [("qk", 3, qt) for qt in range(4)])],
        [],
        [],
    ]
    for t in range(4):
        feeds[t] = list(feeds[t])

    # ---- Attention: one flat software pipeline over (t, qb, kt).
    # AV + denominator run one kt behind scores/exp; each block's stage1 is
    # emitted right after its last AV, stage2 at cycle 1 of the next block.
    blocks = [(t, qb) for t in range(4) for qb in range(NQB)]
    # AV/denominator work runs AVLAG kts behind scores/exp so its exp
    # dependency is long-satisfied when the PE reaches it (no PE idle gap
    # between scores(k+1) and AV(k)).
    AVLAG = 4
    pending = []    # deque of (t, kt, avdA, avdB, ptA, ptB, start, fin, qb)
    norm_q = []     # deferred normalize stage-2 states

    def flush_pending(limit):
        while len(pending) > limit:
            flush_one(pending.pop(0))

    def flush_one(p):
        t, kt, avdA, avdB, ptA, ptB, st, fin, qb = p
        nc.tensor.matmul(avdA[0:64, 0:QB], v_sb[0:64, kt, 2 * t, :],
                         ptA[0:64, :], start=st, stop=fin)
        nc.tensor.matmul(avdA[64:128, 0:QB], v_sb[0:64, kt, 2 * t + 1, :],
                         ptB[0:64, :], start=st, stop=fin)
        nc.tensor.matmul(avdB[0:64, 0:QB], v_sb[64:128, kt, 2 * t, :],
                         ptA[64:128, :], start=st, stop=fin)
        nc.tensor.matmul(avdB[64:128, 0:QB], v_sb[64:128, kt, 2 * t + 1, :],
                         ptB[64:128, :], start=st, stop=fin)
        nc.tensor.matmul(avdA[0:33, QB:2 * QB], ones_sb[0:64, :],
                         ptA[0:64, :], start=st, stop=fin)
        nc.tensor.matmul(avdA[64:97, QB:2 * QB], ones_sb[0:64, :],
                         ptB[0:64, :], start=st, stop=fin)
        nc.tensor.matmul(avdB[0:33, QB:2 * QB], ones_sb[64:128, :],
                         ptA[64:128, :], start=st, stop=fin)
        nc.tensor.matmul(avdB[64:97, QB:2 * QB], ones_sb[64:128, :],
                         ptB[64:128, :], start=st, stop=fin)
        if fin:
            norm_q.append(normalize_stage1(t, qb, avdA, avdB))

    def run_pairs(pair_list, sp_sets, feed_pool):
        for t in pair_list:
            for qb in range(NQB):
                q0 = qb * QB
                avdA = ps_avd.tile([128, 2 * QB], F32, tag="avdA",
                                   name="avdA_t")
                avdB = ps_avd.tile([128, 2 * QB], F32, tag="avdB",
                                   name="avdB_t")
                for kt in range(KTT):
                    k0 = kt * 128
                    cyc = qb * KTT + kt
                    # stage2 lags two blocks so the DRAM bounce of the
                    # denominators (a ~2000-descriptor transposing gather,
                    # 10us+) never head-of-line-blocks the DVE queue.
                    if cyc % KTT == 8 and len(norm_q) >= 2:
                        normalize_stage2(norm_q.pop(0))
                    pool_s, ta, tb = sp_sets[kt % len(sp_sets)]
                    spA = pool_s.tile([128, 512], F32, tag=ta, name="spA_t")
                    spB = pool_s.tile([128, 512], F32, tag=tb, name="spB_t")
                    # scores: even head -> spA (keys on partitions), odd ->
                    # spB
                    nc.tensor.matmul(
                        spA[0:64, :], qk_sb[0:64, 4 + t, k0:k0 + 64],
                        qk_sb[0:64, t, q0:q0 + 512], start=True, stop=True)
                    nc.tensor.matmul(
                        spA[64:128, :], qk_sb[0:64, 4 + t, k0 + 64:k0 + 128],
                        qk_sb[0:64, t, q0:q0 + 512], start=True, stop=True)
                    nc.tensor.matmul(
                        spB[0:64, :], qk_sb[64:128, 4 + t, k0:k0 + 64],
                        qk_sb[64:128, t, q0:q0 + 512], start=True, stop=True)
                    nc.tensor.matmul(
                        spB[64:128, :], qk_sb[64:128, 4 + t, k0 + 64:k0 + 128],
                        qk_sb[64:128, t, q0:q0 + 512], start=True, stop=True)
                    ptA = ppool.tile([128, 512], BF16, tag="ptA", name="ptA_t")
                    ptB = ppool.tile([128, 512], BF16, tag="ptB", name="ptB_t")
                    # exp: ACT does the even head (true exp), the DVE fast-
                    # exp does the odd head.  One instruction per engine per
                    # kt - the per-instruction overheads are large.  The
                    # assignment is fixed so every (query, head) softmax row
                    # is built from one consistent approximation.
                    nc.scalar.activation(ptA, spA, EXP, scale=SCALE)
                    nc.vector.tensor_scalar(
                        ptB[:].bitcast(I16), spB[:],
                        EXPA, EXPB, op0=MULT, op1=ADD)
                    pending.append((t, kt, avdA, avdB, ptA, ptB,
                                    kt == 0, kt == KTT - 1, qb))
                    flush_pending(AVLAG)
                    while feeds[t] and feeds[t][0][0] <= cyc:
                        _, g = feeds[t].pop(0)
                        nfeed[0] += 1
                        tag = "wA" if nfeed[0] % 2 == 0 else "wB"
                        on_act = nfeed[0] % 2 == 1
                        if g[0] == "qk":
                            emit_qk(g[1], g[2], feed_pool, tag, on_act)
                        else:
                            emit_v(g[1], feed_pool, tag, on_act)

    # Pairs 0-1 carry ALL the feed work (their kt cycles are PE-bound, so
    # the exp chain hides inside the feed streams); the feed PSUM pool is
    # scoped so its banks are reclaimed afterwards.
    nfeed = [0]
    with tc.tile_pool(name="ps_w", bufs=1, space="PSUM") as ps_w:
        emit_qk(4, 0, ps_w, "wA")
        emit_qk(0, 0, ps_w, "wB", on_act=True)
        run_pairs([0, 1], [(ps_s, "spA", "spB")], ps_w)

    # Pairs 2-3 are feed-free and would be serialized on exp with a single
    # score-PSUM pair; the two banks freed above double-buffer the scores
    # (alternate bank pairs by kt parity) so exp(kt) overlaps scores(kt+1).
    with tc.tile_pool(name="ps_s2", bufs=1, space="PSUM") as ps_s2:
        run_pairs([2, 3],
                  [(ps_s, "spA", "spB"), (ps_s2, "spA2", "spB2")], None)
        flush_pending(0)
        while norm_q:
            normalize_stage2(norm_q.pop(0))

        # ---- Tail: ALL output-projection groups, full 128x128 mode (no
        # PSUM pair needed, one mode switch total), 4-deep psum rotation so
        # group i+1's matmuls overlap group i's evacuation, copies
        # alternating between ACT and DVE.
        tail = [(mt, et) for mt in range(KTT) for et in range(2)]
        tail_pools = [(ps_s, "spA"), (ps_s2, "spA2"),
                      (ps_s, "spB"), (ps_s2, "spB2")]
        for i, (mt, et) in enumerate(tail):
            pool, tag = tail_pools[i % 4]
            emit_proj(mt, et, pool, tag, on_act=(i % 2 == 1))


def build_nc():
    nc = bacc.Bacc()
    xt = nc.declare_dram_parameter("xt", [128, KT, N], BF16, isOutput=False)
    wqk = nc.declare_dram_parameter("wqk", [128, KT, 2 * FPC], BF16, isOutput=False)
    wv = nc.declare_dram_parameter("wv", [128, KT, FPC], BF16, isOutput=False)
    wp = nc.declare_dram_parameter("wp", [128, 4, D], BF16, isOutput=False)
    bqk = nc.declare_dram_parameter("bqk", [128, 8], F32, isOutput=False)
    out = nc.declare_dram_parameter("out", [N, D], F32, isOutput=True)
    with tile.TileContext(nc) as tc:
        _attn_body(tc, xt, wqk, wv, wp, bqk, out)
    nc.finalize()
    return nc


BF = ml_dtypes.bfloat16


def prep_core_inputs(x, qkv_w, qkv_b, proj_w, c):
    """Build the per-core input map (numpy, final SBUF layouts)."""
    b, hg = divmod(c, 2)
    f0 = hg * FPC
    xt = np.ascontiguousarray(x[b].T)                     # [1024, 2048] f32
    xt_sb = xt.reshape(KT, 128, N).transpose(1, 0, 2)     # [128, 8, 2048]
    wq = qkv_w[f0:f0 + FPC]
    wk = qkv_w[D + f0:D + f0 + FPC]
    wqk = np.concatenate([wq, wk], axis=0)                # [1024, 1024]
    wqk_sb = wqk.T.reshape(KT, 128, 2 * FPC).transpose(1, 0, 2)
    wv = qkv_w[2 * D + f0:2 * D + f0 + FPC]               # [512, 1024]
    wv_sb = wv.T.reshape(KT, 128, FPC).transpose(1, 0, 2)
    wp = proj_w[:, f0:f0 + FPC]                           # [1024e, 512f]
    wp_sb = wp.T.reshape(4, 128, D).transpose(1, 0, 2)
    bqk = np.concatenate(
        [qkv_b[f0:f0 + FPC], qkv_b[D + f0:D + f0 + FPC]]).reshape(8, 128).T
    return {
        "xt": np.ascontiguousarray(xt_sb).astype(BF),
        "wqk": np.ascontiguousarray(wqk_sb).astype(BF),
        "wv": np.ascontiguousarray(wv_sb).astype(BF),
        "wp": np.ascontiguousarray(wp_sb).astype(BF),
        "bqk": np.ascontiguousarray(bqk).astype(np.float32),
    }


def expected_core_out(x, qkv_w, qkv_b, proj_w, c):
    """Numpy model of one core's partial output (for sim debugging)."""
    b, hg = divmod(c, 2)
    f0 = hg * FPC
    xb = x[b].astype(np.float32)
    q = xb @ qkv_w[f0:f0 + FPC].T + qkv_b[f0:f0 + FPC]
    k = xb @ qkv_w[D + f0:D + f0 + FPC].T + qkv_b[D + f0:D + f0 + FPC]
    v = xb @ qkv_w[2 * D + f0:2 * D + f0 + FPC].T          # v-bias folded on host
    out = np.zeros((N, D), np.float32)
    for h in range(HPC):
        qs = q[:, h * HD:(h + 1) * HD]
        ks = k[:, h * HD:(h + 1) * HD]
        vs = v[:, h * HD:(h + 1) * HD]
        s = (qs @ ks.T) * SCALE
        p = np.exp(s - s.max(axis=1, keepdims=True))
        p /= p.sum(axis=1, keepdims=True)
        out += (p @ vs) @ proj_w[:, f0 + h * HD:f0 + (h + 1) * HD].T
    return out


_NC_CACHE = {}


def kernel(x, qkv_w, qkv_b, proj_w, proj_b):
    from concourse.bass_utils import run_bass_kernel_spmd

    x = np.asarray(x, dtype=np.float32)
    qkv_w = np.asarray(qkv_w, dtype=np.float32)
    qkv_b = np.asarray(qkv_b, dtype=np.float32)
    proj_w = np.asarray(proj_w, dtype=np.float32)
    proj_b = np.asarray(proj_b, dtype=np.float32)

    if "nc" not in _NC_CACHE:
        _NC_CACHE["nc"] = build_nc()
    nc = _NC_CACHE["nc"]

    in_maps = [
        prep_core_inputs(x, qkv_w, qkv_b, proj_w, c) for c in range(NCORES)
    ]
    res = run_bass_kernel_spmd(nc, in_maps, core_ids=list(range(NCORES)))
    outs = res.results

    # v-bias folds into a constant row added to every token: proj_w @ v_bias.
    const_row = proj_w @ qkv_b[2 * D:3 * D] + proj_b
    full = np.empty((B, N, D), np.float32)
    for b in range(B):
        full[b] = outs[2 * b]["out"] + outs[2 * b + 1]["out"] + const_row
    return full


# revision 56
# speedup vs baseline: 1.1280x; 1.0877x over previous
"""Multi-head attention (B=4, N=2048, D=1024, H=16) on 8 TRN2 NeuronCores.

Sharding: core c = (batch b = c // 2, head-group hg = c % 2). Each core:
  - computes Q/K/V for its 8 heads (tensor-parallel slice of qkv_w),
  - runs attention for those heads,
  - computes a partial output projection against its 512 columns of proj_w.
Host sums the two partials per batch and adds biases folded on the host.

The ENTIRE kernel runs in the PE's 64x64 array-packing mode (4 concurrent
tiles T0/T2/T8/T10), so HD=64 attention matmuls get ~2x PE throughput and
there are no tiling-mode switches (which drain the array).  bass derives
tile_position from (lhsT.base_partition, out.base_partition) and tile_size
from the AP shapes, so a matmul is a 64x64 tile op iff its lhsT/rhs use 64
partitions and its out covers <=64 partitions.

Device layouts (feature-on-partition; scores come out as S^T [k, q]):
  xt  [128, 8, 2048]  bf16 : x[b]^T, d = kt*128 + p
  wqk [128, 8, 1024]  bf16 : Q (slots 0..3) and K (slots 4..7) weights; the
                             slot t covers head pair (2t, 2t+1): partition
                             range 0:64 = even head dims, 64:128 = odd.
  wv  [128, 8, 512]   bf16 : rhs for V (token-on-partition orientation)
  wp  [128, 4, 1024]  bf16 : lhsT-side contraction layout for the proj
  bqk [128, 8]        f32  : per-feature q/k bias (zero in practice)
  out [2048, 1024]    f32  : partial projection output

Attention per (pair t, q-block qb, key-tile kt), all 64x64 tile ops:
  scores: 4 tiles (head x key-half) -> spA = even S^T [128k, 512q],
          spB = odd S^T.  K=64 per tile (the head dim) -> full PE use.
  exp:    ACT does a fixed 448-column slice (true exp); the DVE does the
          remaining 576 columns with a Schraudolph int16 fast-exp (writing
          the bf16 bit pattern of exp directly).  The column split is fixed
          across kt, so every (query, head) row is normalized by a
          denominator built from the same approximation - the systematic
          part of the fast-exp error cancels in the softmax division.
  AV:     4 tiles (head x k-half) accumulating over kt into avA (k-half 0)
          and avB (k-half 1); summed at evacuation.
  denom:  4 tiles multiplying a [64, 33] ones-column lhsT (M=33 keeps
          tile_size at 64x64) against the same P^T tiles, accumulating the
          softmax denominators in rows 0 / 64 of dA/dB.

Normalization: denominators bounce through DRAM (2 accumulating DMAs), come
back as a 128-partition broadcast, and gpsimd divides (avA+avB) by them into
o_sb.  No DVE reciprocal anywhere near the critical path; the DVE only does
exp plus the proj-group evacuations.

Feed work (QKV projections, V tiles, output-proj groups) is split 4-ways
into the same 64x64 mode (T0/T2/T8/T10 with contraction halves into a psA /
psB pair, summed+biased by gpsimd at evacuation) and issued into the
attention stream by the same (cycle, group) deadline schedule as before.

PSUM budget (8 banks): sp pair 2 + av pair 2 + denom pair 2 + feed pair 2.
The single-buffered score pair works because exp(kt) finishes (ACT 448 cols
~ 430ns in parallel with DVE 576 cols ~ 350ns) before the PE finishes the
rest of the kt cycle (AV + denom, ~430ns), so scores(kt+1) is never gated.
"""

import numpy as np
import ml_dtypes

import concourse.tile as tile
from concourse import bacc, mybir
from concourse._compat import with_exitstack

B, N, D, H, HD = 4, 2048, 1024, 16, 64
NCORES = 8
HPC = 8          # heads per core
FPC = HPC * HD   # 512 features per core
KT = 8           # d-contraction tiles of 128
KTT = 16         # key-token tiles of 128
QB = 512         # q-block size
NQB = N // QB
SCALE = HD ** -0.5

F32 = mybir.dt.float32
BF16 = mybir.dt.bfloat16
I16 = mybir.dt.int16
EXP = mybir.ActivationFunctionType.Exp
IDENT = mybir.ActivationFunctionType.Identity
ADD = mybir.AluOpType.add
MULT = mybir.AluOpType.mult
DIV = mybir.AluOpType.divide

# Schraudolph fast-exp on the DVE: bf16 bit pattern of exp(x*SCALE) is
# approximately round(A*x + B) as int16 (linear-mantissa approximation,
# ~1.8% rms / ~4.2% max relative error at the rms-optimal B).
EXPA = 128.0 * SCALE / float(np.log(2.0))   # 23.0835
EXPB = 16248.5
# Of each kt's 1024 P^T columns (512 even + 512 odd head), the ACT engine
# computes this many with the real exp; the DVE fast-exp does the rest.
ACT_COLS = 448


@with_exitstack
def _attn_body(ctx, tc, xt_d, wqk_d, wv_d, wp_d, bqk_d, out_d):
    nc = tc.nc

    singles = ctx.enter_context(tc.tile_pool(name="singles", bufs=1))
    evac = ctx.enter_context(tc.tile_pool(name="evac", bufs=4))
    ppool = ctx.enter_context(tc.tile_pool(name="ppool", bufs=5))
    rpool = ctx.enter_context(tc.tile_pool(name="rpool", bufs=3))
    dpool = ctx.enter_context(tc.tile_pool(name="dpool", bufs=8, space="DRAM"))
    ps_s = ctx.enter_context(tc.tile_pool(name="ps_s", bufs=1, space="PSUM"))
    ps_avd = ctx.enter_context(tc.tile_pool(name="ps_avd", bufs=1, space="PSUM"))

    # Warm the ACT exp table at t~0 so the ~2.7us ACT_TABLE_LOAD overlaps
    # the input DMAs instead of delaying the first real exp.
    warm = singles.tile([1, 1], F32, name="act_warm")
    nc.vector.memset(warm, 0.0)
    nc.scalar.activation(warm, warm, EXP)

    # Resident SBUF tensors.  wqk first (gates the first K/Q projections),
    # split into two kt-halves so the first projection matmuls start after
    # 1MB; then x token-quarters so dependent work starts per-quarter.
    wqk_h = []
    for h in range(2):
        t = singles.tile([128, KT // 2, 2 * FPC], BF16, name=f"wqk_h{h}")
        wqk_h.append(t)
    nc.sync.dma_start(wqk_h[0], wqk_d[:, 0:KT // 2, :])
    xt_q = []
    for q in range(4):
        t = singles.tile([128, KT, 512], BF16, name=f"xt_q{q}")
        xt_q.append(t)
    nc.sync.dma_start(xt_q[0], xt_d[:, :, 0:512])
    nc.sync.dma_start(wqk_h[1], wqk_d[:, KT // 2:KT, :])
    bqk_sb = singles.tile([128, 8], F32)
    nc.sync.dma_start(bqk_sb, bqk_d[:])
    wv_sb = singles.tile([128, KT, FPC], BF16)
    nc.sync.dma_start(wv_sb, wv_d[:])
    for q in range(1, 4):
        nc.sync.dma_start(xt_q[q], xt_d[:, :, 512 * q:512 * (q + 1)])
    wp_sb = singles.tile([128, 4, D], BF16)
    nc.sync.dma_start(wp_sb, wp_d[:])

    def wqk_kt(kt, f0, width):
        return wqk_h[kt // 4][:, kt % 4, f0:f0 + width]

    def xt_tok(kt, c0, width):
        """Token-range slice of x^T across the quarters (never spans)."""
        q, off = divmod(c0, 512)
        assert off + width <= 512
        return xt_q[q][:, kt, off:off + width]

    qk_sb = singles.tile([128, 8, N], BF16)        # Q^T slots 0..3, K^T slots 4..7
    v_sb = singles.tile([128, KTT, HPC, HD], BF16)  # V, token-on-partition
    o_sb = singles.tile([128, 4, N], BF16)         # normalized attn out
    # ones-column lhsT for the softmax-denominator matmuls: col 0 is 1.0,
    # cols 1..32 are 0 (M=33 so the tile_size stays 64x64).
    ones_sb = singles.tile([128, 33], BF16, name="ones33")
    nc.vector.memset(ones_sb, 0.0)
    nc.vector.memset(ones_sb[:, 0:1], 1.0)

    def emit_qk(ft, qt, pool, tag, on_act=False):
        """One (ft, qt) group of the Q/K projection, full 128x128 mode: 8
        accumulating K=128 matmuls into one PSUM bank + a single bias-fused
        evacuation straight into qk_sb."""
        ps = pool.tile([128, 512], F32, tag=tag, name="qk_ps")
        f0 = ft * 128
        for kt in range(KT):
            nc.tensor.matmul(ps, wqk_kt(kt, f0, 128),
                             xt_tok(kt, qt * 512, 512),
                             start=(kt == 0), stop=(kt == KT - 1))
        dst = qk_sb[:, ft, qt * 512:(qt + 1) * 512]
        if on_act:
            nc.scalar.activation(dst, ps, IDENT, bias=bqk_sb[:, ft:ft + 1])
        else:
            nc.vector.tensor_scalar(dst, ps, bqk_sb[:, ft:ft + 1], None,
                                    op0=ADD)

    def emit_v(mt, pool, tag, on_act=False):
        """One token-tile of the V projection, full 128x128 mode, with a
        single strided evacuation into v_sb."""
        ps = pool.tile([128, 512], F32, tag=tag, name="v_ps")
        for kt in range(KT):
            nc.tensor.matmul(ps, xt_tok(kt, mt * 128, 128),
                             wv_sb[:, kt, :],
                             start=(kt == 0), stop=(kt == KT - 1))
        src = ps.rearrange("p (h e) -> p h e", h=HPC)
        if on_act:
            nc.scalar.copy(v_sb[:, mt, :, :], src)
        else:
            nc.vector.tensor_copy(v_sb[:, mt, :, :], src)

    def emit_proj(mt, et, pool, tag, on_act=False):
        """One output-projection group (tail only, full 128x128 mode): 4
        accumulating K=128 matmuls into a single PSUM bank + copy + store."""
        ps = pool.tile([128, 512], F32, tag=tag, name="pj_ps")
        m0 = mt * 128
        e0 = et * 512
        order = (3, 0, 1, 2)
        for i, t4 in enumerate(order):
            nc.tensor.matmul(ps, o_sb[:, t4, m0:m0 + 128],
                             wp_sb[:, t4, e0:e0 + 512],
                             start=(i == 0), stop=(i == 3))
        ot = evac.tile([128, 512], F32, tag="oevac", name="o_evac")
        if on_act:
            nc.scalar.copy(ot, ps)
        else:
            nc.vector.tensor_copy(ot, ps)
        nc.sync.dma_start(out_d[m0:m0 + 128, e0:e0 + 512], ot)

    # Normalization.  stage1 (at block end): exit the merged AV+denominator
    # pair from PSUM with one big copy per tile (DVE for A, ACT for B) and
    # bounce the 4 denominator partial rows through DRAM into a 128-lane
    # layout.  stage2 (mid next block): reciprocal, broadcast back, and
    # gpsimd combines (afA + afB) * (1/d) into o_sb.
    def normalize_stage1(t, qb, avdA, avdB):
        afA = rpool.tile([128, 2 * QB], F32, tag="afA", name="afA_t")
        afB = rpool.tile([128, 2 * QB], F32, tag="afB", name="afB_t")
        # four half-size exits split across ACT and DVE so the banks free
        # ~2x sooner (the next block's AV start is gated on them)
        nc.vector.tensor_copy(afA[:, 0:QB], avdA[:, 0:QB])
        nc.scalar.copy(afA[:, QB:2 * QB], avdA[:, QB:2 * QB])
        nc.vector.tensor_copy(afB[:, QB:2 * QB], avdB[:, QB:2 * QB])
        nc.scalar.copy(afB[:, 0:QB], avdB[:, 0:QB])
        # 4 cheap row DMAs out; the partition broadcasts back are 64
        # contiguous descriptors each (NO transposing gathers - those cost
        # ~13us of sync-sequencer descriptor generation and saturated it)
        rd = dpool.tile([4, QB], F32, name="d_dram")
        nc.sync.dma_start(rd[0:1, :], afA[0:1, QB:2 * QB])
        nc.sync.dma_start(rd[1:2, :], afA[64:65, QB:2 * QB])
        nc.sync.dma_start(rd[2:3, :], afB[0:1, QB:2 * QB])
        nc.sync.dma_start(rd[3:4, :], afB[64:65, QB:2 * QB])
        dqA = rpool.tile([128, QB], F32, tag="dqA", name="dqA_t")
        dqB = rpool.tile([128, QB], F32, tag="dqB", name="dqB_t")
        nc.sync.dma_start(dqA[0:64, :], rd[0:1, :].partition_broadcast(64))
        nc.sync.dma_start(dqA[64:128, :], rd[1:2, :].partition_broadcast(64))
        nc.sync.dma_start(dqB[0:64, :], rd[2:3, :].partition_broadcast(64))
        nc.sync.dma_start(dqB[64:128, :], rd[3:4, :].partition_broadcast(64))
        return (t, qb, afA, afB, dqA, dqB)

    def normalize_stage2(st):
        t, qb, afA, afB, dqA, dqB = st
        q0 = qb * QB
        dq = rpool.tile([128, QB], F32, tag="dq", name="dq_t")
        nc.gpsimd.tensor_add(dq, dqA, dqB)
        # full-width approximate reciprocal: one custom DVE op (~18 correct
        # bits, far beyond bf16 needs)
        rb = rpool.tile([128, QB], F32, tag="rb", name="rb_t")
        nc.vector.reciprocal_approx_fast(rb, dq)
        af = rpool.tile([128, QB], F32, tag="af", name="af_t")
        nc.gpsimd.tensor_add(af, afA[:, 0:QB], afB[:, 0:QB])
        nc.gpsimd.tensor_mul(o_sb[:, t, q0:q0 + QB], af, rb)



    # Deadline-scheduled feeds: per pair, a list of (cycle, group) issued
    # into the attention stream once the (qb*KTT + kt) cycle counter reaches
    # `cycle`.  Deadlines: V(kt) <= kt (AV of block 0 consumes it), K(4,j)
    # <= 4j-1, Q(0,qb) <= 16qb-1, pair-(t+1) slots <= 63.
    feeds = [
        [(0, ("v", 0)), (0, ("v", 1)), (1, ("v", 2)), (2, ("v", 3)),
         (2, ("qk", 4, 1)), (3, ("v", 4)), (4, ("v", 5)), (5, ("v", 6)),
         (6, ("v", 7)), (6, ("qk", 4, 2)), (7, ("v", 8)), (8, ("v", 9)),
         (9, ("v", 10)), (10, ("v", 11)), (10, ("qk", 4, 3)),
         (11, ("v", 12)), (12, ("v", 13)), (13, ("v", 14)),
         (13, ("qk", 0, 1)), (14, ("v", 15)),
         (16, ("qk", 0, 2)), (18, ("qk", 5, 0)), (20, ("qk", 5, 1)),
         (22, ("qk", 5, 2)), (24, ("qk", 5, 3)), (26, ("qk", 1, 0)),
         (28, ("qk", 1, 1)), (30, ("qk", 1, 2)), (32, ("qk", 1, 3)),
         (34, ("qk", 0, 3))],
        [(4 * i, ("qk", f, qt))
         for i, (f, qt) in enumerate((f, qt) for qt in range(4) for f in (2, 6))]
        + [(30 + 4 * i, g) for i, g in enumerate(
            [("qk", 7, qt) for qt in range(4)]
            + [("qk", 3, qt) for qt in range(4)])],
        [],
        [],
    ]
    for t in range(4):
        feeds[t] = list(feeds[t])

    # ---- Attention: one flat software pipeline over (t, qb, kt).
    # AV + denominator run one kt behind scores/exp; each block's stage1 is
    # emitted right after its last AV, stage2 at cycle 1 of the next block.
    blocks = [(t, qb) for t in range(4) for qb in range(NQB)]
    # AV/denominator work runs AVLAG kts behind scores/exp so its exp
    # dependency is long-satisfied when the PE reaches it (no PE idle gap
    # between scores(k+1) and AV(k)).
    AVLAG = 4
    pending = []    # deque of (t, kt, avdA, avdB, ptA, ptB, start, fin, qb)
    norm_q = []     # deferred normalize stage-2 states

    def flush_pending(limit):
        while len(pending) > limit:
            flush_one(pending.pop(0))

    def flush_one(p):
        t, kt, avdA, avdB, ptA, ptB, st, fin, qb = p
        nc.tensor.matmul(avdA[0:64, 0:QB], v_sb[0:64, kt, 2 * t, :],
                         ptA[0:64, :], start=st, stop=fin)
        nc.tensor.matmul(avdA[64:128, 0:QB], v_sb[0:64, kt, 2 * t + 1, :],
                         ptB[0:64, :], start=st, stop=fin)
        nc.tensor.matmul(avdB[0:64, 0:QB], v_sb[64:128, kt, 2 * t, :],
                         ptA[64:128, :], start=st, stop=fin)
        nc.tensor.matmul(avdB[64:128, 0:QB], v_sb[64:128, kt, 2 * t + 1, :],
                         ptB[64:128, :], start=st, stop=fin)
        nc.tensor.matmul(avdA[0:33, QB:2 * QB], ones_sb[0:64, :],
                         ptA[0:64, :], start=st, stop=fin)
        nc.tensor.matmul(avdA[64:97, QB:2 * QB], ones_sb[0:64, :],
                         ptB[0:64, :], start=st, stop=fin)
        nc.tensor.matmul(avdB[0:33, QB:2 * QB], ones_sb[64:128, :],
                         ptA[64:128, :], start=st, stop=fin)
        nc.tensor.matmul(avdB[64:97, QB:2 * QB], ones_sb[64:128, :],
                         ptB[64:128, :], start=st, stop=fin)
        if fin:
            norm_q.append(normalize_stage1(t, qb, avdA, avdB))

    def run_pairs(pair_list, sp_sets, feed_pool):
        for t in pair_list:
            for qb in range(NQB):
                q0 = qb * QB
                avdA = ps_avd.tile([128, 2 * QB], F32, tag="avdA",
                                   name="avdA_t")
                avdB = ps_avd.tile([128, 2 * QB], F32, tag="avdB",
                                   name="avdB_t")
                for kt in range(KTT):
                    k0 = kt * 128
                    cyc = qb * KTT + kt
                    # stage2 lags two blocks so the DRAM bounce of the
                    # denominators (a ~2000-descriptor transposing gather,
                    # 10us+) never head-of-line-blocks the DVE queue.
                    if cyc % KTT == 8 and len(norm_q) >= 2:
                        normalize_stage2(norm_q.pop(0))
                    pool_s, ta, tb = sp_sets[kt % len(sp_sets)]
                    spA = pool_s.tile([128, 512], F32, tag=ta, name="spA_t")
                    spB = pool_s.tile([128, 512], F32, tag=tb, name="spB_t")
                    # scores: even head -> spA (keys on partitions), odd ->
                    # spB
                    nc.tensor.matmul(
                        spA[0:64, :], qk_sb[0:64, 4 + t, k0:k0 + 64],
                        qk_sb[0:64, t, q0:q0 + 512], start=True, stop=True)
                    nc.tensor.matmul(
                        spA[64:128, :], qk_sb[0:64, 4 + t, k0 + 64:k0 + 128],
                        qk_sb[0:64, t, q0:q0 + 512], start=True, stop=True)
                    nc.tensor.matmul(
                        spB[0:64, :], qk_sb[64:128, 4 + t, k0:k0 + 64],
                        qk_sb[64:128, t, q0:q0 + 512], start=True, stop=True)
                    nc.tensor.matmul(
                        spB[64:128, :], qk_sb[64:128, 4 + t, k0 + 64:k0 + 128],
                        qk_sb[64:128, t, q0:q0 + 512], start=True, stop=True)
                    ptA = ppool.tile([128, 512], BF16, tag="ptA", name="ptA_t")
                    ptB = ppool.tile([128, 512], BF16, tag="ptB", name="ptB_t")
                    # exp: ACT does the even head (true exp), the DVE fast-
                    # exp does the odd head.  One instruction per engine per
                    # kt - the per-instruction overheads are large.  The
                    # assignment is fixed so every (query, head) softmax row
                    # is built from one consistent approximation.
                    nc.scalar.activation(ptA, spA, EXP, scale=SCALE)
                    nc.vector.tensor_scalar(
                        ptB[:].bitcast(I16), spB[:],
                        EXPA, EXPB, op0=MULT, op1=ADD)
                    pending.append((t, kt, avdA, avdB, ptA, ptB,
                                    kt == 0, kt == KTT - 1, qb))
                    flush_pending(AVLAG)
                    while feeds[t] and feeds[t][0][0] <= cyc:
                        _, g = feeds[t].pop(0)
                        nfeed[0] += 1
                        tag = "wA" if nfeed[0] % 2 == 0 else "wB"
                        on_act = nfeed[0] % 2 == 1
                        if g[0] == "qk":
                            emit_qk(g[1], g[2], feed_pool, tag, on_act)
                        else:
                            emit_v(g[1], feed_pool, tag, on_act)

    # Pairs 0-1 carry ALL the feed work (their kt cycles are PE-bound, so
    # the exp chain hides inside the feed streams); the feed PSUM pool is
    # scoped so its banks are reclaimed afterwards.
    nfeed = [0]
    with tc.tile_pool(name="ps_w", bufs=1, space="PSUM") as ps_w:
        emit_qk(4, 0, ps_w, "wA")
        emit_qk(0, 0, ps_w, "wB", on_act=True)
        run_pairs([0, 1], [(ps_s, "spA", "spB")], ps_w)

    # Pairs 2-3 are feed-free and would be serialized on exp with a single
    # score-PSUM pair; the two banks freed above double-buffer the scores
    # (alternate bank pairs by kt parity) so exp(kt) overlaps scores(kt+1).
    with tc.tile_pool(name="ps_s2", bufs=1, space="PSUM") as ps_s2:
        run_pairs([2, 3],
                  [(ps_s, "spA", "spB"), (ps_s2, "spA2", "spB2")], None)
        flush_pending(0)
        while norm_q:
            normalize_stage2(norm_q.pop(0))

        # ---- Tail: ALL output-projection groups, full 128x128 mode (no
        # PSUM pair needed, one mode switch total), 4-deep psum rotation so
        # group i+1's matmuls overlap group i's evacuation, copies
        # alternating between ACT and DVE.
        tail = [(mt, et) for mt in range(KTT) for et in range(2)]
        tail_pools = [(ps_s, "spA"), (ps_s2, "spA2"),
                      (ps_s, "spB"), (ps_s2, "spB2")]
        for i, (mt, et) in enumerate(tail):
            pool, tag = tail_pools[i % 4]
            emit_proj(mt, et, pool, tag, on_act=(i % 2 == 1))


def build_nc():
    nc = bacc.Bacc()
    xt = nc.declare_dram_parameter("xt", [128, KT, N], BF16, isOutput=False)
    wqk = nc.declare_dram_parameter("wqk", [128, KT, 2 * FPC], BF16, isOutput=False)
    wv = nc.declare_dram_parameter("wv", [128, KT, FPC], BF16, isOutput=False)
    wp = nc.declare_dram_parameter("wp", [128, 4, D], BF16, isOutput=False)
    bqk = nc.declare_dram_parameter("bqk", [128, 8], F32, isOutput=False)
    out = nc.declare_dram_parameter("out", [N, D], F32, isOutput=True)
    with tile.TileContext(nc) as tc:
        _attn_body(tc, xt, wqk, wv, wp, bqk, out)
    nc.finalize()
    return nc


BF = ml_dtypes.bfloat16


def prep_core_inputs(x, qkv_w, qkv_b, proj_w, c):
    """Build the per-core input map (numpy, final SBUF layouts)."""
    b, hg = divmod(c, 2)
    f0 = hg * FPC
    xt = np.ascontiguousarray(x[b].T)                     # [1024, 2048] f32
    xt_sb = xt.reshape(KT, 128, N).transpose(1, 0, 2)     # [128, 8, 2048]
    wq = qkv_w[f0:f0 + FPC]
    wk = qkv_w[D + f0:D + f0 + FPC]
    wqk = np.concatenate([wq, wk], axis=0)                # [1024, 1024]
    wqk_sb = wqk.T.reshape(KT, 128, 2 * FPC).transpose(1, 0, 2)
    wv = qkv_w[2 * D + f0:2 * D + f0 + FPC]               # [512, 1024]
    wv_sb = wv.T.reshape(KT, 128, FPC).transpose(1, 0, 2)
    wp = proj_w[:, f0:f0 + FPC]                           # [1024e, 512f]
    wp_sb = wp.T.reshape(4, 128, D).transpose(1, 0, 2)
    bqk = np.concatenate(
        [qkv_b[f0:f0 + FPC], qkv_b[D + f0:D + f0 + FPC]]).reshape(8, 128).T
    return {
        "xt": np.ascontiguousarray(xt_sb).astype(BF),
        "wqk": np.ascontiguousarray(wqk_sb).astype(BF),
        "wv": np.ascontiguousarray(wv_sb).astype(BF),
        "wp": np.ascontiguousarray(wp_sb).astype(BF),
        "bqk": np.ascontiguousarray(bqk).astype(np.float32),
    }


def expected_core_out(x, qkv_w, qkv_b, proj_w, c):
    """Numpy model of one core's partial output (for sim debugging)."""
    b, hg = divmod(c, 2)
    f0 = hg * FPC
    xb = x[b].astype(np.float32)
    q = xb @ qkv_w[f0:f0 + FPC].T + qkv_b[f0:f0 + FPC]
    k = xb @ qkv_w[D + f0:D + f0 + FPC].T + qkv_b[D + f0:D + f0 + FPC]
    v = xb @ qkv_w[2 * D + f0:2 * D + f0 + FPC].T          # v-bias folded on host
    out = np.zeros((N, D), np.float32)
    for h in range(HPC):
        qs = q[:, h * HD:(h + 1) * HD]
        ks = k[:, h * HD:(h + 1) * HD]
        vs = v[:, h * HD:(h + 1) * HD]
        s = (qs @ ks.T) * SCALE
        p = np.exp(s - s.max(axis=1, keepdims=True))
        p /= p.sum(axis=1, keepdims=True)
        out += (p @ vs) @ proj_w[:, f0 + h * HD:f0 + (h + 1) * HD].T
    return out


_NC_CACHE = {}


def kernel(x, qkv_w, qkv_b, proj_w, proj_b):
    from concourse.bass_utils import run_bass_kernel_spmd

    x = np.asarray(x, dtype=np.float32)
    qkv_w = np.asarray(qkv_w, dtype=np.float32)
    qkv_b = np.asarray(qkv_b, dtype=np.float32)
    proj_w = np.asarray(proj_w, dtype=np.float32)
    proj_b = np.asarray(proj_b, dtype=np.float32)

    if "nc" not in _NC_CACHE:
        _NC_CACHE["nc"] = build_nc()
    nc = _NC_CACHE["nc"]

    in_maps = [
        prep_core_inputs(x, qkv_w, qkv_b, proj_w, c) for c in range(NCORES)
    ]
    res = run_bass_kernel_spmd(nc, in_maps, core_ids=list(range(NCORES)))
    outs = res.results

    # v-bias folds into a constant row added to every token: proj_w @ v_bias.
    const_row = proj_w @ qkv_b[2 * D:3 * D] + proj_b
    full = np.empty((B, N, D), np.float32)
    for b in range(B):
        full[b] = outs[2 * b]["out"] + outs[2 * b + 1]["out"] + const_row
    return full
